# revision 1
# baseline (speedup 1.0000x reference)
"""Bass/Trainium2 kernel for the 2-layer GAT model (nn_GATModel).

Strategy (8-core SPMD, edge/graph parallelism):
  - Host: pad N to NPAD = CORES*NB*128; sort edges by dst; partition nodes
    contiguously across cores (each core owns NB node-blocks of 128); group
    each block's in-edges into CMAX chunks of 128 edges (zero-padded).
  - Device, per core:
    Phase Z  (replicated): z_aug = x @ [W1 | W1@Al | W1@Ar]  -> DRAM table
             [NPAD, 272] = rows [z(256) | el(8) | er(8)].
    Phase E1 (edge phase, own nodes only): per chunk, indirect-gather
             z_aug[src] rows and er=z_aug[dst][264:272]; ex =
             exp(leaky_relu(el_s + er_d)); rhs = [zs*ex_bcast | ex]; one-hot
             matmul  G_chunk^T @ rhs  accumulates [rst_unnorm | denom] in
             PSUM per node block.  Normalize per node, + b1, ELU -> h1.
             (softmax max-subtraction is skipped: mathematically invariant)
    z2_aug = h1 @ [W2 | W2@al2 | W2@ar2]  (PE transposes + tiny matmuls)
    AllGather the per-core z2_aug slices ([NPAD,4] total, ~800KB).
    Phase E2: same edge structure, gather 16B rows of the z2_aug table for
             src+dst, flipped matmul (lhsT = per-edge values [128,3],
             rhs = G) accumulates [msg | denom] transposed [3, 128] in PSUM.
             Normalize -> output [nodes, 2].
"""

import sys, os, time
sys.path.insert(0, "/opt/trn_rl_repo")

import numpy as np
from dataclasses import dataclass, field

from concourse import bass, bacc, mybir, tile
from concourse import bass_utils

P = 128


@dataclass
class Cfg:
    N: int = 50000
    E: int = 800000
    IN: int = 128
    H: int = 8
    F1: int = 32
    C: int = 2
    neg_slope: float = 0.2
    cores: int = 8
    NB: int = 49              # node blocks per core
    CMAX: int = 18            # chunks (of 128 edges) per node block
    GRP1: int = 1             # node blocks per L1 gather group
    GRP2: int = 1             # node blocks per L2 gather group
    # dtype knobs
    z_dt: object = mybir.dt.bfloat16    # z_aug table dtype
    g_dt: object = mybir.dt.bfloat16    # one-hot G dtype
    dst_via_gt: bool = True             # dst-side values via GT matmul
    g_on_device: bool = False           # build G on device via iota-compare
    debug_outs: bool = False
    body_reps: int = 1                  # emit the body N times (timing slope)

    @property
    def HF(self):
        return self.H * self.F1         # 256

    @property
    def ZW(self):
        return self.HF + 2 * self.H     # 272 (z | el | er)

    @property
    def NPAD(self):
        return self.cores * self.NB * P

    @property
    def NCORE(self):
        return self.NB * P              # nodes per core


# ----------------------------------------------------------------------------
# Host-side preprocessing
# ----------------------------------------------------------------------------

def _np_dt(dt):
    import ml_dtypes
    if dt == mybir.dt.float32:
        return np.float32
    if dt == mybir.dt.bfloat16:
        return ml_dtypes.bfloat16
    raise ValueError(dt)


def choose_cmax(dst, cfg: Cfg) -> int:
    blk = np.minimum(dst, cfg.NPAD - 1) // P
    cnt = np.bincount(blk, minlength=cfg.cores * cfg.NB)
    return max(1, int(np.ceil(cnt.max() / P)))


def build_host_data(inputs: dict, cfg: Cfg):
    """Returns (in_maps, meta) for run_bass_kernel_spmd."""
    x = np.asarray(inputs["x"], np.float32)
    src = np.asarray(inputs["src"], np.int64)
    dst = np.asarray(inputs["dst"], np.int64)
    W1 = np.asarray(inputs["W1"], np.float32)
    al1 = np.asarray(inputs["attn_l1"], np.float32)
    ar1 = np.asarray(inputs["attn_r1"], np.float32)
    b1 = np.asarray(inputs["b1"], np.float32)
    W2 = np.asarray(inputs["W2"], np.float32)
    al2 = np.asarray(inputs["attn_l2"], np.float32)
    ar2 = np.asarray(inputs["attn_r2"], np.float32)
    b2 = np.asarray(inputs["b2"], np.float32)

    H, F1, C, HF = cfg.H, cfg.F1, cfg.C, cfg.HF

    # --- augmented weights -------------------------------------------------
    Al = np.zeros((HF, H), np.float32)
    Ar = np.zeros((HF, H), np.float32)
    for h in range(H):
        Al[h * F1:(h + 1) * F1, h] = al1[h]
        Ar[h * F1:(h + 1) * F1, h] = ar1[h]
    Waug = np.concatenate([W1, W1 @ Al, W1 @ Ar], axis=1)     # [IN, 272]

    W2aug = np.concatenate([W2, W2 @ al2.reshape(C, 1),
                            W2 @ ar2.reshape(C, 1)], axis=1)  # [HF, 4]

    # --- xT (padded, transposed) ------------------------------------------
    xpad = np.zeros((cfg.NPAD, cfg.IN), np.float32)
    xpad[:cfg.N] = x
    xT = np.ascontiguousarray(xpad.T)                         # [IN, NPAD]

    # --- edge sort & per-core partition ------------------------------------
    order = np.argsort(dst, kind="stable")
    s_src = src[order]
    s_dst = dst[order]

    nb_tot = cfg.cores * cfg.NB
    CM = cfg.CMAX
    # per (global block) edge segments
    blk_of_edge = s_dst // P
    starts = np.searchsorted(blk_of_edge, np.arange(nb_tot))
    ends = np.searchsorted(blk_of_edge, np.arange(nb_tot) + 1)

    in_maps = []
    g_np_dt = _np_dt(cfg.g_dt)
    for c in range(cfg.cores):
        # per-core padded edge arrays [NB, CMAX*128]
        e_src = np.zeros((cfg.NB, CM * P), np.int32)
        e_dstloc = np.zeros((cfg.NB, CM * P), np.int32)
        e_valid = np.zeros((cfg.NB, CM * P), bool)
        for b in range(cfg.NB):
            gb = c * cfg.NB + b
            s, e = starts[gb], ends[gb]
            n = e - s
            assert n <= CM * P, f"CMAX too small: block {gb} has {n} edges"
            e_src[b, :n] = s_src[s:e]
            e_dstloc[b, :n] = s_dst[s:e] - gb * P
            e_valid[b, :n] = True

        # chunk layout: edge j of block b -> chunk j//128, lane j%128
        # gather idx tile layout [128(lane), NB*CMAX(chunk)]
        src_l = e_src.reshape(cfg.NB, CM, P)        # [b, c, lane]
        dstloc_l = e_dstloc.reshape(cfg.NB, CM, P)
        valid_l = e_valid.reshape(cfg.NB, CM, P)
        dstglob_l = dstloc_l + (np.arange(cfg.NB) * P + c * cfg.NB * P)[:, None, None]
        dstglob_l = np.where(valid_l, dstglob_l, 0)

        # transposed for contiguous per-partition DMA: [128, NB*CMAX]
        sidx_t = np.ascontiguousarray(
            src_l.reshape(cfg.NB * CM, P).T).astype(np.int32)

        m = {
            "xT": xT,
            "Waug": Waug,
            "W2aug": W2aug,
            "b1t": np.broadcast_to(b1, (P, HF)).copy(),
            "b2t": np.broadcast_to(b2, (P, C)).copy(),
            "sidx": sidx_t,
        }
        if not cfg.dst_via_gt:
            m["didx"] = np.ascontiguousarray(
                dstglob_l.reshape(cfg.NB * CM, P).T).astype(np.int32)
        else:
            # global node id per (lane, block) for block-row gathers
            m["nodeid"] = np.ascontiguousarray(
                (np.arange(P)[:, None] + 128 * np.arange(cfg.NB)[None, :]
                 + c * cfg.NB * P)).astype(np.int32)

        # one-hot G, partition-major layout [128(lane), NB*CMAX*128]
        G = np.zeros((cfg.NB * CM, P, P), g_np_dt)   # [chunk, lane, node]
        ch = np.repeat(np.arange(cfg.NB * CM), P).reshape(cfg.NB * CM, P)
        lane = np.tile(np.arange(P), (cfg.NB * CM, 1))
        v = valid_l.reshape(cfg.NB * CM, P)
        G[ch[v], lane[v], dstloc_l.reshape(cfg.NB * CM, P)[v]] = 1
        m["G"] = np.ascontiguousarray(
            G.transpose(1, 0, 2).reshape(P, cfg.NB * CM * P))
        if cfg.dst_via_gt:
            # GT: partition = node-local, free = (chunk, lane)
            m["GT"] = np.ascontiguousarray(
                G.transpose(2, 0, 1).reshape(P, cfg.NB * CM * P))

        in_maps.append(m)

    meta = {"order": order}
    return in_maps, meta


# ----------------------------------------------------------------------------
# Device program
# ----------------------------------------------------------------------------

def build_program(cfg: Cfg, debug: bool = False) -> bacc.Bacc:
    nc = bacc.Bacc("TRN2", target_bir_lowering=False, debug=debug,
                   num_devices=cfg.cores, num_swdge_queues=4)
    _qctr = [0]

    def ind_gather(**kw):
        inst = nc.gpsimd.indirect_dma_start(**kw)
        inst.ins.queue = f"qPoolDynamic{_qctr[0] % 4 or ''}"
        _qctr[0] += 1
        return inst
    f32 = mybir.dt.float32
    ZW, HF, H, C, CM, NB = cfg.ZW, cfg.HF, cfg.H, cfg.C, cfg.CMAX, cfg.NB
    NPAD, NCORE = cfg.NPAD, cfg.NCORE
    NCH = NB * CM                  # chunks per core

    # ---- I/O ----
    xT_d = nc.dram_tensor("xT", [cfg.IN, NPAD], f32, kind="ExternalInput")
    Waug_d = nc.dram_tensor("Waug", [cfg.IN, ZW], f32, kind="ExternalInput")
    W2aug_d = nc.dram_tensor("W2aug", [HF, C + 2], f32, kind="ExternalInput")
    b1t_d = nc.dram_tensor("b1t", [P, HF], f32, kind="ExternalInput")
    b2t_d = nc.dram_tensor("b2t", [P, C], f32, kind="ExternalInput")
    sidx_d = nc.dram_tensor("sidx", [P, NCH], mybir.dt.int32, kind="ExternalInput")
    if cfg.dst_via_gt:
        nodeid_d = nc.dram_tensor("nodeid", [P, NB], mybir.dt.int32,
                                  kind="ExternalInput")
        GT_d = nc.dram_tensor("GT", [P, NCH * P], cfg.g_dt, kind="ExternalInput")
    else:
        didx_d = nc.dram_tensor("didx", [P, NCH], mybir.dt.int32, kind="ExternalInput")
    G_d = nc.dram_tensor("G", [P, NCH * P], cfg.g_dt, kind="ExternalInput")
    out_d = nc.dram_tensor("out", [NCORE, C], f32, kind="ExternalOutput")
    if cfg.debug_outs:
        zdbg_d = nc.dram_tensor("zdbg", [NPAD, ZW], cfg.z_dt, kind="ExternalOutput")
        h1dbg_d = nc.dram_tensor("h1dbg", [NCORE, HF], f32, kind="ExternalOutput")
        t2dbg_d = nc.dram_tensor("t2dbg", [NPAD, C + 2], f32, kind="ExternalOutput")
        accdbg_d = nc.dram_tensor("accdbg", [NCORE, HF + H], f32, kind="ExternalOutput")
        rhsdbg_d = nc.dram_tensor("rhsdbg", [NB * CM * P, HF + H], cfg.z_dt, kind="ExternalOutput")
        zseldbg_d = nc.dram_tensor("zseldbg", [NB * CM * P, ZW - H], cfg.z_dt, kind="ExternalOutput")
        erdbg_d = nc.dram_tensor("erdbg", [NB * CM * P, H], f32, kind="ExternalOutput")

    GRP1, GRP2 = cfg.GRP1, cfg.GRP2
    assert NB % GRP1 == 0 and NB % GRP2 == 0

    z_d = nc.dram_tensor("z_tbl", [NPAD, ZW], cfg.z_dt)       # z_aug table
    t2loc_d = nc.dram_tensor("t2loc", [NCORE, C + 2], f32)    # local z2_aug
    t2_d = nc.dram_tensor("t2_tbl", [NPAD, C + 2], f32)       # gathered table

    with tile.TileContext(nc) as tc:
        for _rep in range(cfg.body_reps):

            # ================= Phase Z =================
            with tc.tile_pool(name="zc", bufs=1) as zc, \
                 tc.tile_pool(name="zw", bufs=4) as zw, \
                 tc.tile_pool(name="zp", bufs=4, space="PSUM") as zp:
                waug_t = zc.tile([P, ZW], f32)
                nc.sync.dma_start(out=waug_t[:], in_=Waug_d[:])
                NBT = NPAD // P
                # xT streamed in column-groups of 8 blocks
                XG = 8
                for g in range(NBT // XG + (1 if NBT % XG else 0)):
                    b0 = g * XG
                    nblk = min(XG, NBT - b0)
                    xt_t = zw.tile([P, XG * P], f32, tag="xt")
                    nc.sync.dma_start(
                        out=xt_t[:, :nblk * P],
                        in_=xT_d[:, b0 * P:(b0 + nblk) * P])
                    for j in range(nblk):
                        b = b0 + j
                        pz = zp.tile([P, ZW], f32, tag="pz", space="PSUM")
                        nc.tensor.matmul(
                            out=pz[:], lhsT=xt_t[:, j * P:(j + 1) * P],
                            rhs=waug_t[:], start=True, stop=True)
                        zs = zw.tile([P, ZW], cfg.z_dt, tag="zs")
                        if b % 2 == 0:
                            nc.vector.tensor_copy(out=zs[:], in_=pz[:])
                        else:
                            nc.scalar.copy(out=zs[:], in_=pz[:])
                        nc.sync.dma_start(
                            out=z_d[b * P:(b + 1) * P, :], in_=zs[:])
                        if cfg.debug_outs:
                            nc.sync.dma_start(
                                out=zdbg_d[b * P:(b + 1) * P, :], in_=zs[:])

            tc.strict_bb_all_engine_barrier()

            # ================= Phase E1 =================
            with tc.tile_pool(name="e1c", bufs=1) as e1c, \
                 tc.tile_pool(name="e1g", bufs=4) as e1g, \
                 tc.tile_pool(name="e1w", bufs=4) as e1w, \
                 tc.tile_pool(name="e1p", bufs=3, space="PSUM") as e1p, \
                 tc.tile_pool(name="ep2", bufs=2, space="PSUM") as ep2, \
                 tc.tile_pool(name="tp", bufs=1, space="PSUM") as tp:
                b1_t = e1c.tile([P, HF], f32)
                nc.sync.dma_start(out=b1_t[:], in_=b1t_d[:])
                w2a_t = e1c.tile([P, 2, C + 2], f32)
                nc.sync.dma_start(
                    out=w2a_t[:],
                    in_=W2aug_d[:].rearrange("(k p) c -> p k c", p=P))
                ident = e1c.tile([P, P], f32)
                from concourse.masks import make_identity
                make_identity(nc, ident[:])
                if cfg.dst_via_gt:
                    nid_t = e1c.tile([P, NB], mybir.dt.int32)
                    nc.sync.dma_start(out=nid_t[:], in_=nodeid_d[:])
                    # prefetch all blocks' er rows up-front
                    er_all = e1c.tile([P, NB, H], cfg.z_dt)
                    for b0 in range(NB):
                        ind_gather(
                            out=er_all[:, b0, :], out_offset=None, in_=z_d[:],
                            in_offset=bass.IndirectOffsetOnAxis(
                                ap=nid_t[:, b0:b0 + 1], axis=0),
                            element_offset=ZW - H)

                for g in range(NB // GRP1):
                    ch0 = g * GRP1 * CM            # first chunk of group
                    nch = GRP1 * CM
                    # gather z rows for src (cols 0:264)
                    si_t = e1g.tile([P, nch], mybir.dt.int32, tag="si")
                    nc.sync.dma_start(out=si_t[:], in_=sidx_d[:, ch0:ch0 + nch])
                    zsel = e1g.tile([P, nch, ZW - H], cfg.z_dt, tag="zsel")
                    for q in range(nch):
                        nc.gpsimd.indirect_dma_start(
                            out=zsel[:, q, :], out_offset=None, in_=z_d[:],
                            in_offset=bass.IndirectOffsetOnAxis(
                                ap=si_t[:, q:q + 1], axis=0))
                    g_t = e1g.tile([P, nch, P], cfg.g_dt, tag="g")
                    nc.sync.dma_start(
                        out=g_t[:], in_=G_d[:, ch0 * P:(ch0 + nch) * P])
                    if cfg.dst_via_gt:
                        gt_t = e1g.tile([P, nch, P], cfg.g_dt, tag="gt")
                        nc.sync.dma_start(
                            out=gt_t[:], in_=GT_d[:, ch0 * P:(ch0 + nch) * P])
                    else:
                        di_t = e1g.tile([P, nch], mybir.dt.int32, tag="di")
                        nc.sync.dma_start(out=di_t[:],
                                          in_=didx_d[:, ch0:ch0 + nch])
                        er_t = e1g.tile([P, nch, H], cfg.z_dt, tag="er")
                        for q in range(nch):
                            ind_gather(
                                out=er_t[:, q, :], out_offset=None, in_=z_d[:],
                                in_offset=bass.IndirectOffsetOnAxis(
                                    ap=di_t[:, q:q + 1], axis=0),
                                element_offset=ZW - H)

                    for j in range(GRP1):
                        b = g * GRP1 + j
                        co = j * CM                # chunk offset in group
                        # rhs tile [128, CM, HF+H]: [msg | ex]
                        rhs = e1w.tile([P, CM, HF + H], cfg.z_dt, tag="rhs")
                        ee = e1w.tile([P, CM, H], cfg.z_dt, tag="ee")
                        if cfg.dst_via_gt:
                            # broadcast to edges: er_d[e,h] via GT matmul
                            erd = ep2.tile([P, CM, H], f32, tag="erd",
                                           space="PSUM")
                            for cc in range(CM):
                                nc.tensor.matmul(
                                    out=erd[:, cc, :],
                                    lhsT=gt_t[:, co + cc, :],
                                    rhs=er_all[:, b, :], start=True, stop=True)
                            er_in = erd[:]
                        else:
                            er_in = er_t[:, co:co + CM, :]
                        # e = el_s + er_d
                        nc.vector.tensor_tensor(
                            out=ee[:], in0=zsel[:, co:co + CM, HF:HF + H],
                            in1=er_in, op=mybir.AluOpType.add)
                        # leaky relu: max(0.2*e, e)
                        ee2 = e1w.tile([P, CM, H], cfg.z_dt, tag="ee2")
                        nc.vector.tensor_scalar_mul(
                            out=ee2[:], in0=ee[:], scalar1=cfg.neg_slope)
                        nc.vector.tensor_tensor(
                            out=ee[:], in0=ee2[:], in1=ee[:],
                            op=mybir.AluOpType.max)
                        # ex = exp(e) -> rhs[:, :, HF:]
                        nc.scalar.activation(
                            out=rhs[:, :, HF:HF + H], in_=ee[:],
                            func=mybir.ActivationFunctionType.Exp)
                        # msg = zs * ex_bcast
                        nc.vector.tensor_tensor(
                            out=rhs[:, :, 0:HF].rearrange(
                                "p c (h f) -> p c h f", f=cfg.F1),
                            in0=zsel[:, co:co + CM, 0:HF].rearrange(
                                "p c (h f) -> p c h f", f=cfg.F1),
                            in1=rhs[:, :, HF:HF + H][:, :, :, None].to_broadcast(
                                [P, CM, H, cfg.F1]),
                            op=mybir.AluOpType.mult)
                        if cfg.debug_outs:
                            nc.sync.dma_start(
                                out=rhsdbg_d[b * CM * P:(b + 1) * CM * P, :]
                                    .rearrange("(c p) f -> p c f", p=P),
                                in_=rhs[:])
                            nc.sync.dma_start(
                                out=zseldbg_d[b * CM * P:(b + 1) * CM * P, :]
                                    .rearrange("(c p) f -> p c f", p=P),
                                in_=zsel[:, co:co + CM, :])
                            erdbg_s = e1w.tile([P, CM, H], f32, tag="erdbg_s")
                            nc.vector.tensor_copy(out=erdbg_s[:], in_=er_in)
                            nc.sync.dma_start(
                                out=erdbg_d[b * CM * P:(b + 1) * CM * P, :]
                                    .rearrange("(c p) f -> p c f", p=P),
                                in_=erdbg_s[:])
                        # accumulate [rstU | denom] in PSUM
                        acc = e1p.tile([P, HF + H], f32, tag="acc", space="PSUM")
                        for cc in range(CM):
                            nc.tensor.matmul(
                                out=acc[:],
                                lhsT=g_t[:, co + cc, :],
                                rhs=rhs[:, cc, :],
                                start=(cc == 0), stop=(cc == CM - 1))
                        # ---- normalize + bias + ELU -> h1 ----
                        den = e1w.tile([P, H], f32, tag="den")
                        nc.vector.tensor_scalar_max(
                            out=den[:], in0=acc[:, HF:HF + H], scalar1=1e-30)
                        rec = e1w.tile([P, H], f32, tag="rec")
                        nc.vector.reciprocal(out=rec[:], in_=den[:])
                        rst = e1w.tile([P, HF], f32, tag="rst")
                        nc.vector.tensor_tensor(
                            out=rst[:].rearrange("p (h f) -> p h f", f=cfg.F1),
                            in0=acc[:, 0:HF].rearrange(
                                "p (h f) -> p h f", f=cfg.F1),
                            in1=rec[:, :, None].to_broadcast([P, H, cfg.F1]),
                            op=mybir.AluOpType.mult)
                        nc.vector.tensor_tensor(
                            out=rst[:], in0=rst[:], in1=b1_t[:],
                            op=mybir.AluOpType.add)
                        # ELU: relu(x) + min(exp(x)-1, 0)
                        h1e = e1w.tile([P, HF], f32, tag="h1e")
                        nc.scalar.activation(
                            out=h1e[:], in_=rst[:],
                            func=mybir.ActivationFunctionType.Exp)
                        nc.vector.tensor_scalar(
                            out=h1e[:], in0=h1e[:], scalar1=1.0, scalar2=0.0,
                            op0=mybir.AluOpType.subtract, op1=mybir.AluOpType.min)
                        h1 = e1w.tile([P, HF], f32, tag="h1")
                        nc.vector.tensor_scalar_max(
                            out=h1[:], in0=rst[:], scalar1=0.0)
                        nc.vector.tensor_tensor(
                            out=h1[:], in0=h1[:], in1=h1e[:],
                            op=mybir.AluOpType.add)
                        if cfg.debug_outs:
                            nc.sync.dma_start(
                                out=h1dbg_d[b * P:(b + 1) * P, :], in_=h1[:])
                            accs = e1w.tile([P, HF + H], f32, tag="accs")
                            nc.vector.tensor_copy(out=accs[:], in_=acc[:])
                            nc.sync.dma_start(
                                out=accdbg_d[b * P:(b + 1) * P, :], in_=accs[:])
                        # ---- z2_aug = h1 @ W2aug  (via PE transposes) ----
                        h1T = e1w.tile([P, 2, P], f32, tag="h1T")
                        for k in range(2):
                            ps_t = tp.tile([P, P], f32, tag="pst", space="PSUM")
                            nc.tensor.transpose(
                                out=ps_t[:], in_=h1[:, k * P:(k + 1) * P],
                                identity=ident[:])
                            nc.scalar.copy(out=h1T[:, k, :], in_=ps_t[:])
                        pz2 = tp.tile([C + 2, P], f32, tag="pz2", space="PSUM")
                        for k in range(2):
                            nc.tensor.matmul(
                                out=pz2[:], lhsT=w2a_t[:, k, :],
                                rhs=h1T[:, k, :], start=(k == 0), stop=(k == 1))
                        z2s = e1w.tile([C + 2, P], f32, tag="z2s")
                        nc.vector.tensor_copy(out=z2s[:], in_=pz2[:])
                        # store rows [node, 4] via transposed-view DMA
                        nc.sync.dma_start(
                            out=t2loc_d[b * P:(b + 1) * P, :].rearrange(
                                "n k -> k n"),
                            in_=z2s[:])

            # ================= AllGather =================
            tc.strict_bb_all_engine_barrier()
            nc.gpsimd.collective_compute(
                "AllGather", mybir.AluOpType.bypass,
                replica_groups=[list(range(cfg.cores))],
                ins=[t2loc_d[:].opt()], outs=[t2_d[:].opt()])
            tc.strict_bb_all_engine_barrier()

            if cfg.debug_outs:
                nc.sync.dma_start(out=t2dbg_d[:], in_=t2_d[:])

            # ================= Phase E2 =================
            with tc.tile_pool(name="e2c", bufs=1) as e2c, \
                 tc.tile_pool(name="e2g", bufs=4) as e2g, \
                 tc.tile_pool(name="e2w", bufs=4) as e2w, \
                 tc.tile_pool(name="e2p", bufs=2, space="PSUM") as e2p:
                b2_t = e2c.tile([P, C], f32)
                nc.sync.dma_start(out=b2_t[:], in_=b2t_d[:])
                ident2 = e2c.tile([P, P], f32)
                from concourse.masks import make_identity as _mi2
                _mi2(nc, ident2[:])
                outN = e2c.tile([P, NB, C], f32)
                if cfg.dst_via_gt:
                    nid2_t = e2c.tile([P, NB], mybir.dt.int32)
                    nc.sync.dma_start(out=nid2_t[:], in_=nodeid_d[:])
                    # prefetch all blocks' er2 values, cast once to bf16
                    er2_af = e2c.tile([P, NB, 1], f32)
                    for b0 in range(NB):
                        ind_gather(
                            out=er2_af[:, b0, :], out_offset=None, in_=t2_d[:],
                            in_offset=bass.IndirectOffsetOnAxis(
                                ap=nid2_t[:, b0:b0 + 1], axis=0),
                            element_offset=C + 1)
                    er2_all = e2c.tile([P, NB, 1], cfg.g_dt)
                    nc.vector.tensor_copy(out=er2_all[:], in_=er2_af[:])

                for g in range(NB // GRP2):
                    ch0 = g * GRP2 * CM
                    nch = GRP2 * CM
                    ii_t = e2g.tile([P, nch], mybir.dt.int32, tag="ii")
                    nc.sync.dma_start(out=ii_t[:],
                                      in_=sidx_d[:, ch0:ch0 + nch])
                    t2g = e2g.tile([P, nch, C + 1], f32, tag="t2g")
                    for q in range(nch):
                        ind_gather(
                            out=t2g[:, q, :], out_offset=None, in_=t2_d[:],
                            in_offset=bass.IndirectOffsetOnAxis(
                                ap=ii_t[:, q:q + 1], axis=0))
                    g2_t = e2g.tile([P, nch, P], cfg.g_dt, tag="g2")
                    nc.sync.dma_start(
                        out=g2_t[:], in_=G_d[:, ch0 * P:(ch0 + nch) * P])
                    gt2_t = e2g.tile([P, nch, P], cfg.g_dt, tag="gt2")
                    nc.sync.dma_start(
                        out=gt2_t[:], in_=GT_d[:, ch0 * P:(ch0 + nch) * P])

                    for j in range(GRP2):
                        b = g * GRP2 + j
                        co = j * CM
                        er2d = e2p.tile([P, CM, 1], f32, tag="er2d",
                                        space="PSUM")
                        for cc in range(CM):
                            nc.tensor.matmul(
                                out=er2d[:, cc, :],
                                lhsT=gt2_t[:, co + cc, :],
                                rhs=er2_all[:, b, :], start=True, stop=True)
                        # lhs values tile [128, CM, 3] = [msg2(2) | ex2]
                        lv = e2w.tile([P, CM, C + 1], cfg.g_dt, tag="lv")
                        e2 = e2w.tile([P, CM, 1], f32, tag="e2")
                        nc.vector.tensor_tensor(
                            out=e2[:], in0=t2g[:, co:co + CM, C:C + 1],
                            in1=er2d[:], op=mybir.AluOpType.add)
                        e2b = e2w.tile([P, CM, 1], f32, tag="e2b")
                        nc.vector.tensor_scalar_mul(
                            out=e2b[:], in0=e2[:], scalar1=cfg.neg_slope)
                        nc.vector.tensor_tensor(
                            out=e2[:], in0=e2b[:], in1=e2[:],
                            op=mybir.AluOpType.max)
                        nc.scalar.activation(
                            out=lv[:, :, C:C + 1], in_=e2[:],
                            func=mybir.ActivationFunctionType.Exp)
                        nc.vector.tensor_tensor(
                            out=lv[:, :, 0:C], in0=t2g[:, co:co + CM, 0:C],
                            in1=lv[:, :, C:C + 1].to_broadcast([P, CM, C]),
                            op=mybir.AluOpType.mult)
                        acc2 = e2p.tile([C + 1, P], f32, tag="acc2",
                                        space="PSUM")
                        for cc in range(CM):
                            nc.tensor.matmul(
                                out=acc2[:], lhsT=lv[:, cc, :],
                                rhs=g2_t[:, co + cc, :],
                                start=(cc == 0), stop=(cc == CM - 1))
                        # normalize: out[n, c] = acc2[c, n]/acc2[C, n] + b2[c]
                        # transpose acc2 -> [128 nodes, 3] first
                        a2s = e2w.tile([C + 1, P], f32, tag="a2s")
                        nc.vector.tensor_copy(out=a2s[:], in_=acc2[:])
                        a2p = e2p.tile([P, C + 1], f32, tag="a2p", space="PSUM")
                        nc.tensor.transpose(out=a2p[:], in_=a2s[:],
                                            identity=ident2[:C + 1, :C + 1])
                        den2 = e2w.tile([P, 1], f32, tag="den2")
                        nc.vector.tensor_scalar_max(
                            out=den2[:], in0=a2p[:, C:C + 1], scalar1=1e-30)
                        rec2 = e2w.tile([P, 1], f32, tag="rec2")
                        nc.vector.reciprocal(out=rec2[:], in_=den2[:])
                        nc.vector.tensor_tensor(
                            out=outN[:, b, :], in0=a2p[:, 0:C],
                            in1=rec2[:].to_broadcast([P, C]),
                            op=mybir.AluOpType.mult)
                        nc.vector.tensor_tensor(
                            out=outN[:, b, :], in0=outN[:, b, :],
                            in1=b2_t[:], op=mybir.AluOpType.add)
                # final store: out_d[b*P+p, c] = outN[p, b, c]
                nc.sync.dma_start(
                    out=out_d[:].rearrange("(b p) c -> p b c", p=P),
                    in_=outN[:])

    nc.compile()
    return nc


# ----------------------------------------------------------------------------
# Full pipeline
# ----------------------------------------------------------------------------

_PROGRAM_CACHE = {}


def get_program(cfg: Cfg):
    key = (cfg.N, cfg.E, cfg.cores, cfg.NB, cfg.CMAX, cfg.GRP1, cfg.GRP2,
           str(cfg.z_dt), str(cfg.g_dt), cfg.g_on_device, cfg.debug_outs,
           cfg.body_reps)
    if key not in _PROGRAM_CACHE:
        _PROGRAM_CACHE[key] = build_program(cfg)
    return _PROGRAM_CACHE[key]


def run(inputs: dict, cfg: Cfg = None, verbose=False, return_raw=False):
    if cfg is None:
        cfg = Cfg()
        cfg.CMAX = choose_cmax(np.asarray(inputs["dst"], np.int64), cfg)
    t0 = time.time()
    in_maps, meta = build_host_data(inputs, cfg)
    t1 = time.time()
    nc = get_program(cfg)
    t2 = time.time()
    res = bass_utils.run_bass_kernel_spmd(
        nc, in_maps, core_ids=list(range(cfg.cores)))
    t3 = time.time()
    if verbose:
        print(f"host prep {t1-t0:.2f}s  program {t2-t1:.2f}s  run {t3-t2:.2f}s")
    out = np.concatenate([res.results[c]["out"] for c in range(cfg.cores)],
                         axis=0)
    if return_raw:
        return out[:cfg.N], res
    return out[:cfg.N]


# ============================================================================
# Harness entry point
# ============================================================================

def kernel(**inputs) -> np.ndarray:
    """Full-input GAT forward on 8 NeuronCores. Returns [N, C] float32."""
    cfg = Cfg()
    cfg.CMAX = choose_cmax(np.asarray(inputs["dst"], np.int64), cfg)
    return run(inputs, cfg)


def _trivial_program(cores=8):
    nc = bacc.Bacc("TRN2", target_bir_lowering=False, debug=False,
                   num_devices=cores)
    f32 = mybir.dt.float32
    x_in = nc.dram_tensor("x", [128, 64], f32, kind="ExternalInput")
    out = nc.dram_tensor("out", [128, 64], f32, kind="ExternalOutput")
    with tile.TileContext(nc) as tc:
        with tc.tile_pool(name="c", bufs=1) as cp:
            t = cp.tile([128, 64], f32)
            nc.sync.dma_start(out=t[:], in_=x_in[:])
            nc.scalar.mul(out=t[:], in_=t[:], mul=2.0)
            nc.sync.dma_start(out=out[:], in_=t[:])
    nc.compile()
    return nc


def estimate_hw_time_ns(inputs, iters=30):
    """HW time estimate: steady-state wall time of the jitted kernel program
    with device-resident inputs, minus the same measurement for a trivial
    program (removes the fixed PJRT dispatch overhead)."""
    from timing import time_program  # dev-tree helper
    cfg = Cfg()
    cfg.CMAX = choose_cmax(np.asarray(inputs["dst"], np.int64), cfg)
    in_maps, _ = build_host_data(inputs, cfg)
    nc = get_program(cfg)
    _, t_kernel = time_program(nc, in_maps, iters=iters)
    nc0 = _trivial_program(cfg.cores)
    triv_maps = [{"x": np.zeros((128, 64), np.float32)}
                 for _ in range(cfg.cores)]
    _, t_triv = time_program(nc0, triv_maps, iters=iters)
    print(f"  (kernel wall min {t_kernel['min_s']*1e3:.2f}ms, "
          f"dispatch floor {t_triv['min_s']*1e3:.2f}ms)")
    return max(t_kernel["min_s"] - t_triv["min_s"], 0.0) * 1e9

